# revision 3
# baseline (speedup 1.0000x reference)
"""Adaptive 5x5 per-pixel convolution on 8 TRN2 NeuronCores.

out[b,c,h,w] = sum_{i,j} x[b,c,h+i,w+j] * kernel[b,i*5+j,h,w]

Sharding: data-parallel over batch B=8 -> 1 batch per core.
Per-core shapes: x [64, 260, 260], kernel [25, 256, 256] -> out [64, 256, 256].

Layout on device: partitions = h-rows (blocks of 124/124/8), free dim = w.
Per (channel, block, tap): DVE tensor_tensor mult + add accumulation.
"""

import os
import sys
from contextlib import ExitStack

import numpy as np

sys.path.insert(0, "/opt/trn_rl_repo")

from concourse import bacc, bass, tile  # noqa: E402
from concourse import mybir  # noqa: E402
from concourse.bass_utils import run_bass_kernel_spmd  # noqa: E402

F32 = mybir.dt.float32

C, HP, WP = 64, 260, 260  # padded x per core
KK, H, W = 25, 256, 256
K = 5

# output-row blocks: (h0, nh); x rows used: [h0, h0+nh+4)
BLOCKS = [(0, 124), (124, 124), (248, 8)]

_CACHE = {}


def _build_nc():
    nc = bacc.Bacc(
        "TRN2", target_bir_lowering=False, debug=False, num_devices=8
    )
    x = nc.dram_tensor("x", [C, HP, WP], F32, kind="ExternalInput").ap()
    k = nc.dram_tensor("k", [KK, H, W], F32, kind="ExternalInput").ap()
    out = nc.dram_tensor("out", [C, H, W], F32, kind="ExternalOutput").ap()

    with tile.TileContext(nc) as tc, ExitStack() as ctx:
        kpool = ctx.enter_context(tc.tile_pool(name="kpool", bufs=2))
        xpool = ctx.enter_context(tc.tile_pool(name="xpool", bufs=4))
        ppool = ctx.enter_context(tc.tile_pool(name="ppool", bufs=4))
        apool = ctx.enter_context(tc.tile_pool(name="apool", bufs=4))

        for h0, nh in BLOCKS:
            # kernel taps for this row-block, resident across the c loop
            ktiles = []
            for t in range(KK):
                kt = kpool.tile([nh, W], F32, tag=f"k{t}")
                nc.sync.dma_start(kt[:], k[t, h0 : h0 + nh, :])
                ktiles.append(kt)

            for c in range(C):
                # 5 row-shifted copies of the x slab so every tap reads
                # partitions starting at 0 (engine APs need 32-aligned base)
                xts = []
                for i in range(K):
                    xt = xpool.tile([nh, WP], F32, tag=f"x{i}")
                    nc.sync.dma_start(xt[:], x[c, h0 + i : h0 + i + nh, :])
                    xts.append(xt)

                acc = apool.tile([nh, W], F32)
                for t in range(KK):
                    i, j = t // K, t % K
                    in0 = xts[i][0:nh, j : j + W]
                    if t == 0:
                        nc.vector.tensor_mul(acc[:], in0, ktiles[t][:])
                    else:
                        p = ppool.tile([nh, W], F32)
                        nc.vector.tensor_mul(p[:], in0, ktiles[t][:])
                        nc.vector.tensor_add(acc[:], acc[:], p[:])

                nc.sync.dma_start(out[c, h0 : h0 + nh, :], acc[:])

    nc.compile()
    return nc


def _get_nc():
    if "nc" not in _CACHE:
        _CACHE["nc"] = _build_nc()
    return _CACHE["nc"]


def run(x, kernel, trace=False):
    """x: [8,64,260,260] f32, kernel: [8,25,256,256] f32 -> ([8,64,256,256], exec_ns)."""
    nc = _get_nc()
    x = np.ascontiguousarray(x, dtype=np.float32)
    kernel = np.ascontiguousarray(kernel, dtype=np.float32)
    in_maps = [{"x": x[b], "k": kernel[b]} for b in range(8)]
    res = run_bass_kernel_spmd(nc, in_maps, core_ids=list(range(8)), trace=trace)
    out = np.stack([res.results[b]["out"] for b in range(8)], axis=0)
    return out, res.exec_time_ns


def kernel(**inputs):
    out, _ = run(inputs["x"], inputs["kernel"], trace=False)
    return out


# revision 5
# speedup vs baseline: 3.5608x; 3.5608x over previous
"""Adaptive 5x5 per-pixel convolution on 8 TRN2 NeuronCores.

out[b,c,h,w] = sum_{i,j} x[b,c,h+i,w+j] * kernel[b,i*5+j,h,w]

Sharding: data-parallel over batch B=8 -> 1 batch per core.
Per-core shapes: x [64, 260, 260], kernel [25, 256, 256] -> out [64, 256, 256].

Device layout: partitions = h-rows, free dim = w. Output rows are produced in
three 124-row blocks (the third overlaps the second; duplicate rows get the
same values). Per (channel, block, tap): one bf16 DVE tensor_tensor multiply
of the x slab (row-aligned, partition base 0) with a row-shifted kernel tile;
a TensorE matmul with a shifted-identity stationary S_i both undoes the row
shift and accumulates all 25 tap products in PSUM. ScalarE copies PSUM->SBUF
(f32) and DMA writes the output rows.

Engine APs must start at 32-aligned partitions, so the tap row shift lives in
the kernel-tile DMA (any partition base) + the S_i stationary, never in a
compute-engine read. The tap column shift j stays in the free dim; odd j reads
a one-column-shifted copy of x so bf16 ops keep 4B alignment (DVE 2x mode).
"""

import sys
from contextlib import ExitStack

import ml_dtypes
import numpy as np

sys.path.insert(0, "/opt/trn_rl_repo")

from concourse import bacc, tile  # noqa: E402
from concourse import mybir  # noqa: E402
from concourse.bass_utils import run_bass_kernel_spmd  # noqa: E402

F32 = mybir.dt.float32
BF16 = mybir.dt.bfloat16
BF16_NP = ml_dtypes.bfloat16

C, HP, WP = 64, 260, 260  # padded x per core
KK, H, W = 25, 256, 256
K = 5

# output-row blocks (h0, 124 rows each); block 2 overlaps block 1
BLOCKS = [0, 124, 132]
NH = 124  # out rows per block
XR = 128  # x rows per block tile

_CACHE = {}


def _build_nc():
    nc = bacc.Bacc(
        "TRN2", target_bir_lowering=False, debug=False, num_devices=8
    )
    x = nc.dram_tensor("x", [C, HP, WP], BF16, kind="ExternalInput").ap()
    k = nc.dram_tensor("k", [KK, H, W], BF16, kind="ExternalInput").ap()
    s = nc.dram_tensor("s", [XR, K * XR], BF16, kind="ExternalInput").ap()
    out = nc.dram_tensor("out", [C, H, W], F32, kind="ExternalOutput").ap()

    with tile.TileContext(nc) as tc, ExitStack() as ctx:
        spool = ctx.enter_context(tc.tile_pool(name="spool", bufs=1))
        kpool = ctx.enter_context(tc.tile_pool(name="kpool", bufs=2))
        xpool = ctx.enter_context(tc.tile_pool(name="xpool", bufs=4))
        ppool = ctx.enter_context(tc.tile_pool(name="ppool", bufs=8))
        opool = ctx.enter_context(tc.tile_pool(name="opool", bufs=4))
        mmpool = ctx.enter_context(tc.tile_pool(name="mm", bufs=4, space="PSUM"))

        stile = spool.tile([XR, K * XR], BF16)
        nc.sync.dma_start(stile[:], s[:])

        for h0 in BLOCKS:
            # kernel tap tiles, row-shifted by -i, resident across the c loop
            ktiles = []
            for t in range(KK):
                i = t // K
                kt = kpool.tile([XR, W], BF16, tag=f"k{t}")
                lo = h0 - i  # desired first k row (tile row p holds k row lo+p)
                klo, khi = max(0, lo), min(H, lo + XR)
                if klo > lo or khi < lo + XR:
                    # partial tile: zero first so unused rows can't inject
                    # NaN*0 into the matmul contraction
                    nc.vector.memset(kt[:], 0.0)
                nc.sync.dma_start(kt[klo - lo : khi - lo, :], k[t, klo:khi, :])
                ktiles.append(kt)

            for c in range(C):
                xt = xpool.tile([XR, WP], BF16, tag="xe")
                nc.sync.dma_start(xt[:], x[c, h0 : h0 + XR, :])
                # one-column-left-shifted copy for odd-j taps (4B alignment)
                xo = xpool.tile([XR, 258], BF16, tag="xo")
                nc.vector.tensor_copy(xo[:], xt[:, 1:259])

                ps = mmpool.tile([NH, W], F32)
                for t in range(KK):
                    i, j = t // K, t % K
                    if j % 2 == 0:
                        in0 = xt[:, j : j + W]
                    else:
                        in0 = xo[:, j - 1 : j - 1 + W]
                    p = ppool.tile([XR, W], BF16)
                    nc.vector.tensor_mul(p[:], in0, ktiles[t][:])
                    nc.tensor.matmul(
                        ps[:],
                        stile[:, i * XR : i * XR + NH],
                        p[:],
                        start=(t == 0),
                        stop=(t == KK - 1),
                    )

                ot = opool.tile([NH, W], F32)
                nc.scalar.copy(ot[:], ps[:])
                nc.sync.dma_start(out[c, h0 : h0 + NH, :], ot[:])

    nc.compile()
    return nc


def _get_nc():
    if "nc" not in _CACHE:
        _CACHE["nc"] = _build_nc()
    return _CACHE["nc"]


def _s_const():
    # S_i[h', h] = 1 iff h' == h + i   (lhsT: out[h,w] = sum_h' S[h',h] P[h',w])
    s = np.zeros((K, XR, XR), dtype=BF16_NP)
    for i in range(K):
        s[i] = np.eye(XR, XR, -i, dtype=np.float32).astype(BF16_NP)
    # host layout [XR, K*XR] so the device tile is [partitions, K*128]
    return np.ascontiguousarray(s.transpose(1, 0, 2).reshape(XR, K * XR))


def run(x, kernel, trace=False):
    """x: [8,64,260,260] f32, kernel: [8,25,256,256] f32 -> ([8,64,256,256], exec_ns)."""
    nc = _get_nc()
    xb = np.asarray(x).astype(BF16_NP)
    kb = np.asarray(kernel).astype(BF16_NP)
    sc = _s_const()
    in_maps = [{"x": xb[b], "k": kb[b], "s": sc} for b in range(8)]
    res = run_bass_kernel_spmd(nc, in_maps, core_ids=list(range(8)), trace=trace)
    out = np.stack([res.results[b]["out"] for b in range(8)], axis=0)
    return out, res.exec_time_ns


def kernel(**inputs):
    out, _ = run(inputs["x"], inputs["kernel"], trace=False)
    return out


# revision 10
# speedup vs baseline: 5.6624x; 1.5902x over previous
"""Adaptive 5x5 per-pixel convolution on 8 TRN2 NeuronCores.

out[b,c,h,w] = sum_{i,j} x[b,c,h+i,w+j] * kernel[b,i*5+j,h,w]

Sharding: data-parallel over batch B=8 -> 1 batch per core.
Per-core: x [64, 260, 260], kernel [25, 256, 256] -> out [64, 256, 256].

Device layout: partitions = h-rows, free = (channel-group, w). Rows 0..247 in
two 124-row blocks over 4-channel groups; rows 248..255 "tail-packed" as
(channel, row) pairs on partitions (10 channels x 12 x-rows = 120 partitions).

Per (block, channel-group, tap-row i): two bf16 DVE tensor_tensor multiplies
(even-j taps from x, odd-j taps from a one-column-shifted copy, keeping the
4B alignment the DVE 2x mode needs) write all five tap products into one
product tile. TensorE matmuls with a shifted-identity stationary S_i undo the
tap row shift and accumulate all 25 taps into PSUM ([124, 512] = 2 channels
per matmul). ScalarE copies PSUM->SBUF f32; DMA stores.

Engine APs must start at 32-aligned partitions, so row shifts live in the
kernel-tile DMA (any partition base) + the stationary, never in a compute read.

Host-side: cast to bf16 and relayout x -> [h, c, w], kernel -> [i, h, j, w],
out <- [h, c, w] so DMA moves >=2KB contiguous runs per partition.
"""

import sys
from contextlib import ExitStack

import ml_dtypes
import numpy as np

sys.path.insert(0, "/opt/trn_rl_repo")

from concourse import bacc, bass, tile  # noqa: E402
from concourse import mybir  # noqa: E402
from concourse.bass_utils import run_bass_kernel_spmd  # noqa: E402

F32 = mybir.dt.float32
BF16 = mybir.dt.bfloat16
BF16_NP = ml_dtypes.bfloat16

C, HP, WP = 64, 260, 260
KK, H, W = 25, 256, 256
K = 5

BLOCKS = [0, 124]
NH = 124  # out rows per main block
XR = 128  # x rows per main tile
CG = 4  # channels per x/product tile
# product j-slot order: evens {0,2,4} then odds {1,3}
SLOT_OF_J = {0: 0, 2: 1, 4: 2, 1: 3, 3: 4}

# tail: out rows 248..255 from x rows 248..259
TH0, TXR, TNH = 248, 12, 8
TCG = 10  # channels per tail group
TGROUPS = [(0, 10), (10, 10), (20, 10), (30, 10), (40, 10), (50, 10), (60, 4)]
TP = TCG * TXR  # 120 tail partitions
TQ = TCG * TNH  # 80 tail psum partitions

_CACHE = {}


def _ap(t, off, dims):
    return bass.AP(t[:].tensor, off, dims)


def _build_nc():
    nc = bacc.Bacc(
        "TRN2", target_bir_lowering=False, debug=False, num_devices=8
    )
    x = nc.dram_tensor("x", [HP, C, WP], BF16, kind="ExternalInput").ap()
    k = nc.dram_tensor("k", [K, H, K, W], BF16, kind="ExternalInput").ap()
    s = nc.dram_tensor("s", [XR, K * XR], BF16, kind="ExternalInput").ap()
    st = nc.dram_tensor("st", [TP, K * TQ], BF16, kind="ExternalInput").ap()
    out = nc.dram_tensor("out", [H, C, W], F32, kind="ExternalOutput").ap()

    with tile.TileContext(nc) as tc, ExitStack() as ctx:
        spool = ctx.enter_context(tc.tile_pool(name="spool", bufs=1))
        kpool = ctx.enter_context(tc.tile_pool(name="kpool", bufs=2))
        ktpool = ctx.enter_context(tc.tile_pool(name="ktpool", bufs=1))
        xpool = ctx.enter_context(tc.tile_pool(name="xpool", bufs=3))
        ppool = ctx.enter_context(tc.tile_pool(name="ppool", bufs=4))
        opool = ctx.enter_context(tc.tile_pool(name="opool", bufs=4))
        mmpool = ctx.enter_context(tc.tile_pool(name="mm", bufs=2, space="PSUM"))

        stile = spool.tile([XR, K * XR], BF16)
        nc.sync.dma_start(stile[:], s[:])
        sttile = spool.tile([TP, K * TQ], BF16)
        nc.sync.dma_start(sttile[:], st[:])

        # ---------------- main blocks ----------------
        for h0 in BLOCKS:
            # k tiles per tap-row i: [128, 5j*256]; row p holds k row h0+p-i
            ktiles = []
            for i in range(K):
                kt = kpool.tile([XR, K * W], BF16, tag=f"k{i}")
                lo = h0 - i
                klo, khi = max(0, lo), min(H, lo + XR)
                if klo > lo or khi < lo + XR:
                    nc.vector.memset(kt[:], 0.0)
                nc.sync.dma_start(
                    kt[klo - lo : khi - lo, :].rearrange(
                        "p (j w) -> p j w", j=K
                    ),
                    k[i, klo:khi, :, :],
                )
                ktiles.append(kt)

            for c0 in range(0, C, CG):
                xt = xpool.tile([XR, CG * WP], BF16, tag="xe")
                nc.sync.dma_start(
                    xt[:].rearrange("p (c w) -> p c w", c=CG),
                    x[h0 : h0 + XR, c0 : c0 + CG, :],
                )
                # one-column-left-shifted copy for odd-j taps
                xo = xpool.tile([XR, CG * WP], BF16, tag="xo")
                nc.vector.tensor_copy(
                    _ap(xo, 0, [[CG * WP, XR], [WP, CG], [1, 258]]),
                    _ap(xt, 1, [[CG * WP, XR], [WP, CG], [1, 258]]),
                )

                # psum accumulators, one per channel pair, live across all i
                pss = [
                    mmpool.tile([NH, 2 * W], F32, tag=f"ps{cp}", name=f"ps{cp}")
                    for cp in range(CG // 2)
                ]
                for i in range(K):
                    kt = ktiles[i]
                    # products [128, (4c)(5slot)(256w)] bf16, fresh per i
                    p = ppool.tile([XR, CG * K * W], BF16, tag="p")
                    # even j {0,2,4} -> slots 0..2
                    nc.vector.tensor_mul(
                        _ap(p, 0, [[CG * K * W, XR], [K * W, CG], [W, 3], [1, W]]),
                        _ap(xt, 0, [[CG * WP, XR], [WP, CG], [2, 3], [1, W]]),
                        _ap(kt, 0, [[K * W, XR], [0, CG], [2 * W, 3], [1, W]]),
                    )
                    # odd j {1,3} -> slots 3..4
                    nc.vector.tensor_mul(
                        _ap(p, 3 * W, [[CG * K * W, XR], [K * W, CG], [W, 2], [1, W]]),
                        _ap(xo, 0, [[CG * WP, XR], [WP, CG], [2, 2], [1, W]]),
                        _ap(kt, W, [[K * W, XR], [0, CG], [2 * W, 2], [1, W]]),
                    )

                    for cp in range(CG // 2):
                        for j in range(K):
                            slot = SLOT_OF_J[j]
                            mv = _ap(
                                p,
                                cp * 2 * K * W + slot * W,
                                [[CG * K * W, XR], [K * W, 2], [1, W]],
                            )
                            nc.tensor.matmul(
                                pss[cp][:],
                                stile[:, i * XR : i * XR + NH],
                                mv,
                                start=(i == 0 and j == 0),
                                stop=(i == K - 1 and j == K - 1),
                            )

                # psum -> sbuf -> dram (per channel pair)
                for cp in range(CG // 2):
                    ot = opool.tile([NH, 2 * W], F32)
                    nc.scalar.copy(ot[:], pss[cp][:])
                    nc.sync.dma_start(
                        out[h0 : h0 + NH, c0 + 2 * cp : c0 + 2 * cp + 2, :],
                        ot[:].rearrange("p (c w) -> p c w", c=2),
                    )

        # ---------------- tail (out rows 248..255) ----------------
        # k tail tiles: [120, 5j*256]; partition (c*12+r) holds k row 248+r-i
        ktt = []
        for i in range(K):
            kt = ktpool.tile([TP, K * W], BF16, tag=f"kt{i}")
            nc.vector.memset(kt[:], 0.0)
            for c in range(TCG):
                nc.sync.dma_start(
                    kt[c * TXR + i : c * TXR + i + TNH, :].rearrange(
                        "p (j w) -> p j w", j=K
                    ),
                    k[i, TH0 : TH0 + TNH, :, :],
                )
            ktt.append(kt)

        pairs = [(0, 1), (2, 3), (4, 5), (6,)]
        for pair in pairs:
            xts, xos = [], []
            for gi, g in enumerate(pair):
                gc0, gnc = TGROUPS[g]
                np_ = gnc * TXR
                xt = xpool.tile([TP, WP], BF16, tag=f"xt{gi}")
                # src x_r[248:260, gc0:gc0+gnc, :] iterated (c, r, w)
                nc.sync.dma_start(
                    _ap(xt, 0, [[WP, np_], [1, WP]]),
                    bass.AP(
                        x.tensor,
                        (TH0 * C + gc0) * WP,
                        [[WP, gnc], [C * WP, TXR], [1, WP]],
                    ),
                )
                xo = xpool.tile([TP, WP], BF16, tag=f"xo{gi}")
                nc.vector.tensor_copy(
                    _ap(xo, 0, [[WP, np_], [1, 258]]),
                    _ap(xt, 1, [[WP, np_], [1, 258]]),
                )
                xts.append((xt, np_))
                xos.append((xo, np_))

            ngr = len(pair)
            qp = TQ if ngr == 2 else TGROUPS[pair[0]][1] * TNH
            pp = TP if ngr == 2 else TGROUPS[pair[0]][1] * TXR
            ps = mmpool.tile([TQ, 2 * W], F32, tag="pst")
            for i in range(K):
                kt = ktt[i]
                pt = ppool.tile([TP, 2 * K * W], BF16, tag="pt")
                for gi in range(ngr):
                    xt, np_ = xts[gi]
                    xo, _ = xos[gi]
                    goff = gi * K * W
                    nc.vector.tensor_mul(
                        _ap(pt, goff, [[2 * K * W, np_], [W, 3], [1, W]]),
                        _ap(xt, 0, [[WP, np_], [2, 3], [1, W]]),
                        _ap(kt, 0, [[K * W, np_], [2 * W, 3], [1, W]]),
                    )
                    nc.vector.tensor_mul(
                        _ap(pt, goff + 3 * W, [[2 * K * W, np_], [W, 2], [1, W]]),
                        _ap(xo, 0, [[WP, np_], [2, 2], [1, W]]),
                        _ap(kt, W, [[K * W, np_], [2 * W, 2], [1, W]]),
                    )

                for j in range(K):
                    slot = SLOT_OF_J[j]
                    mv = _ap(
                        pt, slot * W, [[2 * K * W, pp], [K * W, ngr], [1, W]]
                    )
                    nc.tensor.matmul(
                        ps[0:qp, 0 : ngr * W],
                        sttile[0:pp, i * TQ : i * TQ + qp],
                        mv,
                        start=(i == 0 and j == 0),
                        stop=(i == K - 1 and j == K - 1),
                    )

            ot = opool.tile([TQ, 2 * W], F32, tag="ott")
            nc.scalar.copy(ot[0:qp, 0 : ngr * W], ps[0:qp, 0 : ngr * W])
            for gi, g in enumerate(pair):
                gc0, gnc = TGROUPS[g]
                for c in range(gnc):
                    nc.sync.dma_start(
                        out[TH0 : TH0 + TNH, gc0 + c, :],
                        ot[c * TNH : (c + 1) * TNH, gi * W : (gi + 1) * W],
                    )

    nc.compile()
    return nc


def _get_nc():
    if "nc" not in _CACHE:
        _CACHE["nc"] = _build_nc()
    return _CACHE["nc"]


def _s_const():
    # S_i[p, h] = 1 iff p == h + i ; layout [XR, K*XR]
    s = np.zeros((K, XR, XR), dtype=np.float32)
    for i in range(K):
        s[i] = np.eye(XR, XR, -i)
    return np.ascontiguousarray(
        s.transpose(1, 0, 2).reshape(XR, K * XR)
    ).astype(BF16_NP)


def _st_const():
    # S_tail_i[(c,r), (c',q)] = 1 iff c==c' and r == q + i ; layout [TP, K*TQ]
    stm = np.zeros((K, TP, TQ), dtype=np.float32)
    for i in range(K):
        for c in range(TCG):
            for q in range(TNH):
                stm[i, c * TXR + q + i, c * TNH + q] = 1.0
    return np.ascontiguousarray(
        stm.transpose(1, 0, 2).reshape(TP, K * TQ)
    ).astype(BF16_NP)


def run(x, kernel, trace=False):
    """x: [8,64,260,260] f32, kernel: [8,25,256,256] f32 -> ([8,64,256,256], exec_ns)."""
    nc = _get_nc()
    xb = np.asarray(x).astype(BF16_NP)
    kb = np.asarray(kernel).astype(BF16_NP)
    sc, stc = _s_const(), _st_const()
    in_maps = []
    for b in range(8):
        xr = np.ascontiguousarray(xb[b].transpose(1, 0, 2))  # [h, c, w]
        kr = np.ascontiguousarray(
            kb[b].reshape(K, K, H, W).transpose(0, 2, 1, 3)
        )  # [i, h, j, w]
        in_maps.append({"x": xr, "k": kr, "s": sc, "st": stc})
    res = run_bass_kernel_spmd(nc, in_maps, core_ids=list(range(8)), trace=trace)
    outs = []
    for b in range(8):
        o = res.results[b]["out"]  # [h, c, w]
        outs.append(o.transpose(1, 0, 2))
    return np.ascontiguousarray(np.stack(outs, axis=0)), res.exec_time_ns


def kernel(**inputs):
    out, _ = run(inputs["x"], inputs["kernel"], trace=False)
    return out
